# revision 5
# baseline (speedup 1.0000x reference)
"""Trainium2 Bass kernel for DDGAttention (N=4, L=1024, D=128, H=12, DQK=DV=16).

Sharding: 8 cores = 4 batch x 2 query-halves of 512. Each core runs dense
512x1024 attention for all 12 heads plus the geometric epilogue; the host
shards inputs / gathers outputs (no collectives).

Math restructuring vs the reference:
 - logits computed transposed [j, i] so E = exp(logits^T) feeds the AV matmul
   directly as the moving operand (no alpha transpose).
 - softmax denominator = ones-column appended to the AV stationary operand.
 - rel_pos aggregation: alpha @ rel_pos = alpha @ pos_CB - pos_CA * rowsum(alpha),
   so the (L, L, 3) tensor is never materialized.
 - no max-subtraction in softmax (logits are O(20); fp32 exp is safe), mask
   enters as a per-key exp bias and a per-query multiplier.
 - K=16 logits matmuls packed 4-per-PE-pass via 32-partition row strips;
   M=20 AV matmuls packed 4-per-pass via column strips.
"""

import numpy as np

import concourse.bass as bass
import concourse.mybir as mybir
from concourse.tile import TileContext
from concourse.masks import make_identity
from concourse import bacc, bass_utils

F32 = mybir.dt.float32
AF = mybir.ActivationFunctionType
ALU = mybir.AluOpType

N, L, D = 4, 1024, 128
H, DQK, DV = 12, 16, 16
NCORES = 8
JB = 8          # key blocks of 128
IC = 4          # query chunks of 128 (per 512-half)
G = 3           # head groups of 4
EPS_LN = 1e-5
INF = 1e5

_compiled = {}


def _bap(ap, free_ap):
    """AP with replaced free dims (for 0-step broadcast reads)."""
    return bass.AP(tensor=ap.tensor, offset=ap.offset, ap=[ap.ap[0]] + free_ap)


def _build():
    nc = bacc.Bacc(trn_type="TRN2")

    # ---- I/O ----------------------------------------------------------
    xkv = nc.dram_tensor("xkv", [128, JB * 128], F32, kind="ExternalInput")
    xq = nc.dram_tensor("xq", [128, IC * 128], F32, kind="ExternalInput")
    pcb = nc.dram_tensor("pcb", [128, JB * 3], F32, kind="ExternalInput")
    pca = nc.dram_tensor("pca", [128, IC * 3], F32, kind="ExternalInput")
    frm = nc.dram_tensor("frm", [128, IC * 9], F32, kind="ExternalInput")
    expb = nc.dram_tensor("expb", [128, JB], F32, kind="ExternalInput")
    mski = nc.dram_tensor("mski", [128, IC], F32, kind="ExternalInput")
    wqp = nc.dram_tensor("wqp", [128, G * 128], F32, kind="ExternalInput")
    wkp = nc.dram_tensor("wkp", [128, G * 128], F32, kind="ExternalInput")
    wv = nc.dram_tensor("wv", [128, H * DV], F32, kind="ExternalInput")
    wo01 = nc.dram_tensor("wo01", [256, 128], F32, kind="ExternalInput")
    wo2 = nc.dram_tensor("wo2", [20, 128], F32, kind="ExternalInput")
    bob = nc.dram_tensor("bob", [128, 128], F32, kind="ExternalInput")
    gmb = nc.dram_tensor("gmb", [128, 128], F32, kind="ExternalInput")
    btb = nc.dram_tensor("btb", [128, 128], F32, kind="ExternalInput")
    out = nc.dram_tensor("out", [IC * 128, 128], F32, kind="ExternalOutput")

    with TileContext(nc) as tc:
        with tc.tile_pool(name="sing", bufs=1) as sing, \
             tc.tile_pool(name="epool", bufs=6) as epool, \
             tc.tile_pool(name="ep", bufs=3) as ep, \
             tc.tile_pool(name="pslg", bufs=3, space="PSUM") as pslg, \
             tc.tile_pool(name="psav", bufs=2, space="PSUM") as psav:

            # ---- load constants / inputs ------------------------------
            ident = sing.tile([128, 128], F32)
            make_identity(nc, ident)
            xkv_sb = sing.tile([128, JB, 128], F32)
            nc.sync.dma_start(out=xkv_sb, in_=xkv[:].rearrange("p (b d) -> p b d", b=JB))
            xq_sb = sing.tile([128, IC, 128], F32)
            nc.sync.dma_start(out=xq_sb, in_=xq[:].rearrange("p (b d) -> p b d", b=IC))
            pcb_sb = sing.tile([128, JB, 3], F32)
            nc.sync.dma_start(out=pcb_sb, in_=pcb[:].rearrange("p (b c) -> p b c", b=JB))
            pca_sb = sing.tile([128, IC, 3], F32)
            nc.sync.dma_start(out=pca_sb, in_=pca[:].rearrange("p (b c) -> p b c", b=IC))
            frm_sb = sing.tile([128, IC, 9], F32)
            nc.sync.dma_start(out=frm_sb, in_=frm[:].rearrange("p (b c) -> p b c", b=IC))
            expb_sb = sing.tile([128, JB], F32)
            nc.sync.dma_start(out=expb_sb, in_=expb[:])
            mski_sb = sing.tile([128, IC], F32)
            nc.sync.dma_start(out=mski_sb, in_=mski[:])
            wqp_sb = sing.tile([128, G * 128], F32)
            nc.sync.dma_start(out=wqp_sb, in_=wqp[:])
            wkp_sb = sing.tile([128, G * 128], F32)
            nc.sync.dma_start(out=wkp_sb, in_=wkp[:])
            wv_sb = sing.tile([128, H * DV], F32)
            nc.sync.dma_start(out=wv_sb, in_=wv[:])
            wo0_sb = sing.tile([128, 128], F32)
            nc.sync.dma_start(out=wo0_sb, in_=wo01[0:128, :])
            wo1_sb = sing.tile([128, 128], F32)
            nc.sync.dma_start(out=wo1_sb, in_=wo01[128:256, :])
            wo2_sb = sing.tile([20, 128], F32)
            nc.sync.dma_start(out=wo2_sb, in_=wo2[:])
            bob_sb = sing.tile([128, 128], F32)
            nc.sync.dma_start(out=bob_sb, in_=bob[:])
            gmb_sb = sing.tile([128, 128], F32)
            nc.sync.dma_start(out=gmb_sb, in_=gmb[:])
            btb_sb = sing.tile([128, 128], F32)
            nc.sync.dma_start(out=btb_sb, in_=btb[:])
            eps_sb = sing.tile([128, 1], F32)
            nc.vector.memset(eps_sb, EPS_LN)

            # ---- phase B: transposes x -> xT --------------------------
            xT = sing.tile([128, L], F32)          # [d, j]
            for jb in range(JB):
                tp = pslg.tile([128, 1024], F32, tag="lg", name="tpx")
                nc.tensor.transpose(tp[:, 0:128], xkv_sb[:, jb, :], ident)
                nc.vector.tensor_copy(xT[:, jb * 128:(jb + 1) * 128], tp[:, 0:128])
            xqT = sing.tile([128, 512], F32)       # [d, i]
            for ic in range(IC):
                tp = pslg.tile([128, 1024], F32, tag="lg", name="tpxq")
                nc.tensor.transpose(tp[:, 0:128], xq_sb[:, ic, :], ident)
                nc.vector.tensor_copy(xqT[:, ic * 128:(ic + 1) * 128], tp[:, 0:128])

            # ---- phase C: qT / kT / v+A' ------------------------------
            qT = []
            kT = []
            for g in range(G):
                qt = sing.tile([128, 512], F32, name=f"qT{g}")
                qp = pslg.tile([128, 1024], F32, tag="lg", name="qps")
                nc.tensor.matmul(qp[:, 0:512], wqp_sb[:, g * 128:(g + 1) * 128], xqT,
                                 start=True, stop=True)
                nc.vector.tensor_copy(qt, qp[:, 0:512])
                qT.append(qt)
                kt = sing.tile([128, L], F32, name=f"kT{g}")
                for hf in range(2):
                    kp = pslg.tile([128, 1024], F32, tag="lg", name="kps")
                    nc.tensor.matmul(kp[:, 0:512], wkp_sb[:, g * 128:(g + 1) * 128],
                                     xT[:, hf * 512:(hf + 1) * 512], start=True, stop=True)
                    nc.vector.tensor_copy(kt[:, hf * 512:(hf + 1) * 512], kp[:, 0:512])
                kT.append(kt)

            # A' packed per key block: [j, h, 20] = [v_h | pos_CB | 1]
            apk = sing.tile([128, JB, H, 20], F32)
            for jb in range(JB):
                vp = pslg.tile([128, 1024], F32, tag="lg", name="vps")
                nc.tensor.matmul(vp[:, 0:H * DV], xT[:, jb * 128:(jb + 1) * 128], wv_sb,
                                 start=True, stop=True)
                nc.vector.tensor_copy(
                    apk[:, jb, :, 0:16],
                    vp[:, 0:H * DV].rearrange("p (h c) -> p h c", c=16))
                nc.vector.tensor_copy(
                    apk[:, jb, :, 16:19],
                    _bap(pcb_sb[:, jb, :], [[0, H], [1, 3]]))
                nc.vector.memset(apk[:, jb, :, 19:20], 1.0)

            # ---- phase D: logits -> exp -> AV -------------------------
            F_sb = []
            for g in range(G):
                av = psav.tile([128, 512], F32, tag="av", name="av")
                nc.vector.memset(av, 0.0)
                for jb in range(JB):
                    lgA = pslg.tile([128, 1024], F32, tag="lg", name="lgA")
                    lgB = pslg.tile([128, 1024], F32, tag="lg", name="lgB")
                    for t in range(4):
                        dst = (lgA, lgB)[t // 2][:, (t % 2) * 512:((t % 2) + 1) * 512]
                        nc.tensor.matmul(
                            dst,
                            kT[g][32 * t:32 * t + 16, jb * 128:(jb + 1) * 128],
                            qT[g][32 * t:32 * t + 16, :],
                            start=True, stop=True, tile_position=(32 * t, 0))
                    eA = epool.tile([128, 1024], F32, tag="E", name="eA")
                    eB = epool.tile([128, 1024], F32, tag="E", name="eB")
                    nc.scalar.activation(out=eA, in_=lgA, func=AF.Exp,
                                         bias=expb_sb[:, jb:jb + 1], scale=1.0)
                    nc.scalar.activation(out=eB, in_=lgB, func=AF.Exp,
                                         bias=expb_sb[:, jb:jb + 1], scale=1.0)
                    for t in range(4):
                        esl = (eA, eB)[t // 2][:, (t % 2) * 512:((t % 2) + 1) * 512]
                        nc.tensor.matmul(
                            av[32 * t:32 * t + 20, :],
                            apk[:, jb, 4 * g + t, :],
                            esl,
                            start=(jb == 0), stop=(jb == JB - 1),
                            tile_position=(0, 32 * t), skip_group_check=True)
                fg = sing.tile([128, 512], F32, name=f"F{g}")
                nc.vector.tensor_copy(fg, av)
                F_sb.append(fg)

            # ---- phase E: epilogue per query chunk --------------------
            for ic in range(IC):
                ft = ep.tile([128, G * 128], F32, tag="ft", name="ft")
                for g in range(G):
                    tp = pslg.tile([128, 1024], F32, tag="lg", name="tpf")
                    nc.tensor.transpose(tp[:, 0:128],
                                        F_sb[g][:, ic * 128:(ic + 1) * 128], ident)
                    nc.vector.tensor_copy(ft[:, g * 128:(g + 1) * 128], tp[:, 0:128])
                # ft cols: g*128 + 32*t + c ; c in [0,20) valid
                ft4 = ft.rearrange("p (g t c) -> p g t c", t=4, c=32)
                msk_ic = mski_sb[:, ic:ic + 1]

                den = ep.tile([128, H, 1], F32, tag="s12", name="den")
                nc.vector.tensor_copy(
                    den.rearrange("p (g t) o -> p g t o", t=4), ft4[:, :, :, 19:20])
                rden = ep.tile([128, H], F32, tag="s12b", name="rden")
                nc.vector.reciprocal(rden, den.rearrange("p h o -> p (h o)"))
                r = ep.tile([128, H], F32, tag="s12c", name="r")
                nc.vector.tensor_scalar_mul(r, rden, msk_ic)

                # S[p, h, c] = F[p, h, c] * r[p, h]  for c in [0,20)
                S = ep.tile([128, H, 20], F32, tag="s240", name="S")
                nc.vector.tensor_mul(
                    S.rearrange("p (g t) c -> p g t c", t=4),
                    ft4[:, :, :, 0:20],
                    _bap(r, [[4, G], [1, 4], [0, 20]]))

                fa0 = ep.tile([128, 128], F32, tag="fa0", name="fa0")
                fa1 = ep.tile([128, 128], F32, tag="fa1", name="fa1")
                fa2 = ep.tile([128, 20], F32, tag="fa2", name="fa2")
                # feat_node rows 0..191
                nc.vector.tensor_copy(
                    fa0.rearrange("p (h c) -> p h c", c=16), S[:, 0:8, 0:16])
                nc.vector.tensor_copy(
                    fa1[:, 0:64].rearrange("p (h c) -> p h c", c=16), S[:, 8:12, 0:16])

                # apb = S[:, :, 16:19] - pos_CA * mask
                pcam = ep.tile([128, 3], F32, tag="s3", name="pcam")
                nc.vector.tensor_scalar_mul(pcam, pca_sb[:, ic, :], msk_ic)
                apb = ep.tile([128, H, 3], F32, tag="s36", name="apb")
                nc.vector.tensor_sub(apb, S[:, :, 16:19],
                                     _bap(pcam, [[0, H], [1, 3]]))

                # distance -> fa1[:, 100:112]  (rows 228..239)
                sq = ep.tile([128, H, 3], F32, tag="s36b", name="sq")
                nc.vector.tensor_mul(sq, apb, apb)
                d2 = ep.tile([128, H], F32, tag="s12d", name="d2")
                nc.vector.reduce_sum(out=d2, in_=sq, axis=mybir.AxisListType.X)
                nc.scalar.activation(out=fa1[:, 100:112], in_=d2, func=AF.Sqrt)

                # feat_points: fp[p,h,a] = sum_b frame[p,a,b] * apb[p,h,b]
                prod = ep.tile([128, H, 3, 3], F32, tag="s108", name="prod")
                nc.vector.tensor_mul(
                    prod,
                    _bap(apb, [[3, H], [0, 3], [1, 3]]),
                    _bap(frm_sb[:, ic, :], [[0, H], [3, 3], [1, 3]]))
                fp = ep.tile([128, H * 3], F32, tag="s36c", name="fp")
                nc.vector.reduce_sum(out=fp.rearrange("p (x a) -> p x a", a=3),
                                     in_=prod.rearrange("p h a b -> p (h a) b"),
                                     axis=mybir.AxisListType.X)
                nc.vector.tensor_copy(fa1[:, 64:100], fp)

                # direction = fp / (|fp| + 1e-10)
                fsq = ep.tile([128, H * 3], F32, tag="s36d", name="fsq")
                nc.vector.tensor_mul(fsq, fp, fp)
                n2 = ep.tile([128, H], F32, tag="s12e", name="n2")
                nc.vector.reduce_sum(out=n2, in_=fsq.rearrange("p (x a) -> p x a", a=3),
                                     axis=mybir.AxisListType.X)
                nrm = ep.tile([128, H], F32, tag="s12f", name="nrm")
                nc.scalar.activation(out=nrm, in_=n2, func=AF.Sqrt)
                nc.vector.tensor_scalar_add(nrm, nrm, 1e-10)
                rn = ep.tile([128, H], F32, tag="s12g", name="rn")
                nc.vector.reciprocal(rn, nrm)
                dire = ep.tile([128, H * 3], F32, tag="s36e", name="dire")
                nc.vector.tensor_mul(dire.rearrange("p (h a) -> p h a", a=3),
                                     fp.rearrange("p (h a) -> p h a", a=3),
                                     _bap(rn, [[1, H], [0, 3]]))
                nc.vector.tensor_copy(fa1[:, 112:128], dire[:, 0:16])
                nc.vector.tensor_copy(fa2, dire[:, 16:36])

                # feat_all^T via transposes, then @ Wo
                wo_ps = psav.tile([128, 512], F32, tag="av", name="wops")
                for cc, (fax, kk) in enumerate([(fa0, 128), (fa1, 128), (fa2, 20)]):
                    tp = pslg.tile([128, 1024], F32, tag="lg", name="tpa")
                    nc.tensor.transpose(tp[0:kk, 0:128], fax, ident)
                    fxt = ep.tile([128, 128], F32, tag="fxt", name="fxt")
                    nc.vector.tensor_copy(fxt[0:kk, :], tp[0:kk, 0:128])
                    rhs = (wo0_sb, wo1_sb, wo2_sb)[cc]
                    nc.tensor.matmul(wo_ps[:, 0:128], fxt[0:kk, :], rhs,
                                     start=(cc == 0), stop=(cc == 2))

                # y = (wo + bo) * mask + xq ; layernorm
                y = ep.tile([128, 128], F32, tag="y", name="y")
                nc.vector.tensor_add(y, wo_ps[:, 0:128], bob_sb)
                nc.vector.tensor_scalar_mul(y, y, msk_ic)
                nc.vector.tensor_add(y, y, xq_sb[:, ic, :])
                st6 = ep.tile([128, 6], F32, tag="st6", name="st6")
                nc.vector.bn_stats(out=st6, in_=y)
                mv = ep.tile([128, 2], F32, tag="mv", name="mv")
                nc.vector.bn_aggr(out=mv, in_=st6)
                std = ep.tile([128, 1], F32, tag="std", name="std")
                nc.scalar.activation(out=std, in_=mv[:, 1:2], func=AF.Sqrt,
                                     bias=eps_sb, scale=1.0)
                rstd = ep.tile([128, 1], F32, tag="rstd", name="rstd")
                nc.vector.reciprocal(rstd, std)
                xc = ep.tile([128, 128], F32, tag="xc", name="xc")
                nc.vector.tensor_scalar(out=xc, in0=y, scalar1=mv[:, 0:1],
                                        scalar2=rstd, op0=ALU.subtract, op1=ALU.mult)
                o1 = ep.tile([128, 128], F32, tag="o1", name="o1")
                nc.vector.tensor_mul(o1, xc, gmb_sb)
                nc.vector.tensor_add(o1, o1, btb_sb)
                nc.sync.dma_start(
                    out=out[:].rearrange("(c p) d -> c p d", p=128)[ic], in_=o1)

    nc.compile()
    return nc


def _pad_heads(w):
    """[128, 192] -> [128, 384]: head 4g+t at columns g*128 + 32t .. +16."""
    o = np.zeros((128, G * 128), np.float32)
    for g in range(G):
        for t in range(4):
            h = 4 * g + t
            o[:, g * 128 + 32 * t: g * 128 + 32 * t + 16] = w[:, h * 16:(h + 1) * 16]
    return o


def _pm(a, nb):
    """[nb*128, F] -> partition-major [128, nb*F]."""
    f = a.shape[-1]
    return np.ascontiguousarray(
        a.reshape(nb, 128, f).transpose(1, 0, 2).reshape(128, nb * f))


def kernel(x, pos_CA, pos_CB, frame, mask, Wq, Wk, Wv, Wo, bo, gamma, beta):
    x = np.asarray(x, np.float32)
    pos_CA = np.asarray(pos_CA, np.float32)
    pos_CB = np.asarray(pos_CB, np.float32)
    frame = np.asarray(frame, np.float32)
    maskf = np.asarray(mask).astype(np.float32)
    Wq = np.asarray(Wq, np.float32)
    Wk = np.asarray(Wk, np.float32)
    Wv = np.asarray(Wv, np.float32)
    Wo = np.asarray(Wo, np.float32)
    bo = np.asarray(bo, np.float32)
    gamma = np.asarray(gamma, np.float32)
    beta = np.asarray(beta, np.float32)

    if "nc" not in _compiled:
        _compiled["nc"] = _build()
    nc = _compiled["nc"]

    wqp = _pad_heads(Wq)
    wkp = _pad_heads(Wk)
    wo01 = np.ascontiguousarray(np.vstack([Wo[0:256, :],]))
    wo2 = np.ascontiguousarray(Wo[256:276, :])
    bob = np.ascontiguousarray(np.tile(bo[None, :], (128, 1)))
    gmb = np.ascontiguousarray(np.tile(gamma[None, :], (128, 1)))
    btb = np.ascontiguousarray(np.tile(beta[None, :], (128, 1)))

    in_maps = []
    for c in range(NCORES):
        n, hf = c // 2, c % 2
        xn = x[n]
        sl = slice(hf * 512, (hf + 1) * 512)
        in_maps.append({
            "xkv": _pm(xn, 8),
            "xq": _pm(xn[sl], 4),
            "pcb": _pm(pos_CB[n], 8),
            "pca": _pm(pos_CA[n, sl], 4),
            "frm": _pm(frame[n, sl].reshape(512, 9), 4),
            "expb": np.ascontiguousarray(
                (-INF * (1.0 - maskf[n])).reshape(8, 128).T),
            "mski": np.ascontiguousarray(maskf[n, sl].reshape(4, 128).T),
            "wqp": wqp, "wkp": wkp, "wv": Wv,
            "wo01": wo01, "wo2": wo2,
            "bob": bob, "gmb": gmb, "btb": btb,
        })

    res = bass_utils.run_bass_kernel_spmd(nc, in_maps, core_ids=list(range(NCORES)))
    full = np.empty((N, L, D), np.float32)
    for c in range(NCORES):
        n, hf = c // 2, c % 2
        full[n, hf * 512:(hf + 1) * 512, :] = res.results[c]["out"]
    return full
